# revision 40
# baseline (speedup 1.0000x reference)
"""Trainium2 Bass kernel for nn_DecentralizedCoordinator (GNN message passing).

Strategy (8 NeuronCores, SPMD, 4 launches). The SWDGE gather descriptor
rate (~7-8 ns/row on the GpSimd Q7, regardless of row size) is the machine
bottleneck for message passing, so the design minimizes gathered rows:

- L-A  logits = x @ w_lead + b (nodes sharded by id, one block-major DMA +
  batched DVE mult/reduce per core).
- host gathers logits into a per-dst padded layout (pure index routing).
- L-B  leader election per dst (reduce_max / is_equal / mult(src+1) /
  reduce_max; exact reference tie-break semantics) -> leader id per node.
- host: only ~31% of nodes are ever somebody's leader and only their
  reports are read. Distinct leaders are re-balanced across cores/blocks;
  only edges into leader dsts (~30k/core instead of ~100k/core) are kept.
  Referenced source nodes are greedily MATCHED INTO PAIRS within dst
  blocks; one 512B dma_gather descriptor fetches a pair-row [x_u | x_v]
  from a per-core compacted table (single int16 subtable), cutting
  descriptors another ~43%. Per-(column,block) occurrence fp8 scatter
  matrices (one per pair half, host-precomputed, exact small ints) route
  each half's edges to dst slots with multiplicity.
- L-C  segment mean + report MLP for leader nodes only: scatter matmuls
  (lhsT = gathered bf16 half, rhs = fp8 matrix) accumulate sums
  TRANSPOSED [feature, dst] in PSUM; per-dst 1/max(cnt,1) (host index
  metadata) applied on DVE; w1 -> gelu(+b1) -> w2 (+b2 on DVE) without any
  on-chip transpose; reports written bf16 transposed.
- host assembles the global leader-report table + per-node positions.
- L-D  out[n] = reports[leader[n]]: same pair-matched gather (position
  pairs co-referenced by an output block share a 512B descriptor) + fp8
  one-hot expansion matmuls into f32 PSUM (cast for free via scalar copy).

Host only shards/reshapes/gathers-by-index between launches; all
arithmetic on values happens on device.  1093us -> ~351us measured.
"""
import os
import sys
import hashlib

import numpy as np
import ml_dtypes

sys.path.insert(0, "/opt/trn_rl_repo")

import concourse.bass as bass
import concourse.tile as tile
from concourse import bacc, mybir
from concourse.bass_utils import run_bass_kernel_spmd
from concourse.masks import make_identity

dt = mybir.dt
bf16 = ml_dtypes.bfloat16

P = 128
NCORES = 8
BPC = 98                 # dst blocks per core (all-nodes layout)
NPC = BPC * P            # 12544 nodes per core
NPAD = NCORES * NPC      # 100352 padded node count
NSUB = 4
SUB = NPAD // NSUB       # 25088 rows per gather subtable
H = 128
C = 128
CW = 32                  # gather-window width (columns of 128 rows)
NEG = -3.0e38
N_NODES = 100000

CORES = list(range(NCORES))


def _wrap_idx16(local_idx):
    """[NI] int array -> [128, NI//16] int16 (16-wrap, replicated x8)."""
    ni = len(local_idx)
    assert ni % 16 == 0
    w = np.asarray(local_idx, np.int16).reshape(ni // 16, 16).T  # [16, NI/16]
    return np.tile(w, (8, 1)).copy()


def _assign_nodes(nodes, indeg4, nblk):
    """Balanced node -> (core, block, slot) over NCORES x nblk x P slots.

    nodes: [M] node ids to place; indeg4: [M, NSUB] per-subtable in-degree.
    Equalizes per-(block, sub) in-degree across cores (vector-aware greedy).
    Returns node2kbp {node: (k, b, p)} arrays and inv [NCORES, nblk, P].
    """
    M = len(nodes)
    assert M <= NCORES * nblk * P
    indeg = indeg4.sum(axis=1)
    order = np.argsort(-indeg, kind="stable")
    n2k = np.zeros(M, np.int64)
    n2b = np.zeros(M, np.int64)
    n2p = np.zeros(M, np.int64)
    inv = np.full((NCORES, nblk, P), -1, np.int64)
    BIG = 1 << 40
    per_blk = NCORES * P
    for b in range(nblk):
        sl = order[b * per_blk : (b + 1) * per_blk]
        loads = np.zeros((NCORES, indeg4.shape[1]), np.float64)
        caps = np.full(NCORES, P, np.int64)
        slots = np.zeros(NCORES, np.int64)
        for j in sl:
            v = indeg4[j]
            cost = ((loads + v) ** 2).sum(axis=1)
            cost[caps == 0] = BIG
            k = int(np.argmin(cost))
            loads[k] += v
            caps[k] -= 1
            p = int(slots[k])
            slots[k] += 1
            n2k[j] = k
            n2b[j] = b
            n2p[j] = p
            inv[k, b, p] = nodes[j]
    return n2k, n2b, n2p, inv


def _pack_paired(edges_per_core, nblk, cw, whole_blocks=False):
    """Paired gather packing. edges_per_core[k] = (src, blk, dstp) per-edge
    arrays (dst owned by core k). Nodes referenced by a core are greedily
    matched into pairs within blocks; a 512B descriptor fetches one pair-row
    [x_u | x_v]; a pair occupies one slot per block it has edges into, and
    per-occurrence fp8 scatter matrices (one per half) route each half's
    edges to their dst slots (with multiplicity).

    Returns shared structure (occs/windows, max over cores) + per-core
    (pairs, idx16, amat fp8, nrows).
    """
    # --- per-core pairing ---------------------------------------------------
    pairs_uv = []
    hit_kb = []          # per core: dict arrays (sorted by (b, pid))
    for k in range(NCORES):
        src, blk, dstp = edges_per_core[k]
        key = src * nblk + blk
        uk = np.unique(key)
        us, ub = uk // nblk, uk % nblk
        order = np.argsort(ub, kind="stable")
        us_s, ub_s = us[order], ub[order]
        bnd = np.searchsorted(ub_s, np.arange(nblk + 1))
        matched = np.zeros(NPAD, bool)
        pu, pv = [], []
        for b in range(nblk):
            nodes = us_s[bnd[b] : bnd[b + 1]]
            nodes = nodes[~matched[nodes]]
            m = (len(nodes) // 2) * 2
            pu.append(nodes[0:m:2])
            pv.append(nodes[1:m:2])
            matched[nodes[:m]] = True
        alln = np.unique(us)
        singles = alln[~matched[alln]]
        if len(singles) % 2:
            singles = np.concatenate([singles, singles[-1:]])
        pu.append(singles[0::2])
        pv.append(singles[1::2])
        pu = np.concatenate(pu).astype(np.int64)
        pv = np.concatenate(pv).astype(np.int64)
        pairs_uv.append((pu, pv))
        pid = np.full(NPAD, -1, np.int64)
        pid[pu] = np.arange(len(pu))
        pid[pv] = np.arange(len(pu))
        # distinct (pair, block) hits, sorted by (b, pid)
        hkey = ub * (NPAD + 1) + pid[us]
        hk = np.unique(hkey)
        hb, hp = hk // (NPAD + 1), hk % (NPAD + 1)
        hit_kb.append((hb, hp, pid))

    # --- shared slot structure ----------------------------------------------
    cnt_kb = np.zeros((NCORES, nblk), np.int64)
    for k in range(NCORES):
        hb = hit_kb[k][0]
        cnt_kb[k] = np.bincount(hb, minlength=nblk)
    m_b = cnt_kb.max(axis=0)
    occs = []
    occ_of_block = [[] for _ in range(nblk)]
    occ_idx = {}
    if whole_blocks:
        # FFD bin-pack: each block's slots sit whole inside one column
        assert m_b.max() <= 128
        col_of_b = np.zeros(nblk, np.int64)
        lo_of_b = np.zeros(nblk, np.int64)
        space = []
        for b in np.argsort(-m_b):
            sz = int(m_b[b])
            if sz == 0:
                continue
            for c in range(len(space)):
                if space[c] >= sz:
                    lo_of_b[b] = 128 - space[c]
                    col_of_b[b] = c
                    space[c] -= sz
                    break
            else:
                col_of_b[b] = len(space)
                lo_of_b[b] = 0
                space.append(128 - sz)
        NCOL = len(space)
        slot_base = col_of_b * 128 + lo_of_b
        order_b = sorted(range(nblk),
                         key=lambda b: (int(col_of_b[b]), int(lo_of_b[b])))
        for b in order_b:
            if m_b[b] == 0:
                continue
            c, lo = int(col_of_b[b]), int(lo_of_b[b])
            occ_idx[(c, b)] = len(occs)
            occ_of_block[b].append(len(occs))
            occs.append((c, b, lo, lo + int(m_b[b])))
        block_order = order_b
    else:
        slot_off = np.concatenate([[0], np.cumsum(m_b)[:-1]])
        slot_base = slot_off
        total = int(m_b.sum())
        NCOL = (total + 127) // 128
        for b in range(nblk):
            s0 = int(slot_off[b])
            s1 = s0 + int(m_b[b])
            if s1 == s0:
                continue
            for c in range(s0 // 128, (s1 - 1) // 128 + 1):
                lo = max(s0, c * 128) - c * 128
                hi = min(s1, (c + 1) * 128) - c * 128
                occ_idx[(c, b)] = len(occs)
                occ_of_block[b].append(len(occs))
                occs.append((c, b, lo, hi))
        block_order = list(range(nblk))
    NOCC = len(occs)

    # windows of cw columns; split the final window in two so the tail
    # (compute that can only start after the last gather) stays small
    bounds = list(range(0, NCOL, cw)) + [NCOL]
    if len(bounds) >= 2 and bounds[-1] - bounds[-2] > cw // 2:
        bounds.insert(-1, (bounds[-2] + bounds[-1] + 1) // 2)
    win_range = {}
    win_of_col = np.zeros(NCOL, np.int64)
    for w in range(len(bounds) - 1):
        win_range[w] = (bounds[w], bounds[w + 1])
        win_of_col[bounds[w] : bounds[w + 1]] = w
    win_occ_range = {}
    for j, (c, b, lo, hi) in enumerate(occs):
        w = int(win_of_col[c])
        if w not in win_occ_range:
            win_occ_range[w] = [j, j + 1]
        else:
            win_occ_range[w][1] = j + 1

    # --- per-core idx + matrices --------------------------------------------
    NI = NCOL * 128
    idx16 = []
    amat = []
    nrows = []
    for k in range(NCORES):
        src, blk, dstp = edges_per_core[k]
        hb, hp, pid = hit_kb[k]
        pu, pv = pairs_uv[k]
        nrows.append(len(pu))
        # global slot of each hit: slot_off[b] + rank within block
        starts = np.searchsorted(hb, np.arange(nblk))
        rank = np.arange(len(hb)) - starts[hb]
        gslot = slot_base[hb] + rank
        idx_all = np.zeros(NI, np.int64)
        idx_all[gslot] = hp
        idx16.append(_wrap_idx16(idx_all))
        # per-edge: locate slot, half, occurrence
        ep = pid[src]
        hkey = blk * (NPAD + 1) + ep
        hpos = np.searchsorted(hb * (NPAD + 1) + hp, hkey)
        es = gslot[hpos]
        half = np.where(src == pu[ep], 0, 1)
        ecol = es // 128
        srow = es % 128
        eocc = np.array([occ_idx[(int(c), int(b))]
                         for c, b in zip(ecol, blk)], np.int64)
        A = np.zeros((P, NOCC, 2, P), np.float32)
        np.add.at(A, (srow, eocc, half, dstp), 1.0)
        amat.append(np.ascontiguousarray(
            A.reshape(P, NOCC * 2 * P)).astype(ml_dtypes.float8_e4m3))
    NROWS = max(nrows)

    return dict(
        occs=occs, occ_of_block=occ_of_block, NOCC=NOCC, NCOL=NCOL,
        win_range=win_range, win_occ_range=win_occ_range,
        idx16=idx16, amat=amat, pairs=pairs_uv, NROWS=NROWS, cw=cw,
        block_order=block_order, win_of_col=win_of_col,
    )


def _preprocess_edges(edge_index):
    """Edge-only preprocessing (independent of values): all-nodes layout,
    election layout."""
    row = np.asarray(edge_index[0], np.int64)
    col = np.asarray(edge_index[1], np.int64)

    # trivial node -> (core, block, slot) by id; last 352 slots empty
    ids = np.arange(NPAD, dtype=np.int64)
    own_k = ids // NPC
    own_b = (ids % NPC) // P
    own_p = ids % P
    inv = np.where(ids < N_NODES, ids, -1).reshape(NCORES, BPC, P)

    # extended-edge (leader) layout: uniform width WU per dst
    cnt = np.bincount(col, minlength=NPAD)
    WU = int(cnt.max()) + 1
    dorder = np.argsort(col, kind="stable")
    row_d = row[dorder]
    d_sorted = col[dorder]
    dst_starts = np.concatenate([[0], np.cumsum(cnt)])
    ranks = np.arange(len(col)) - np.repeat(dst_starts[:-1], cnt)
    elog = np.full((NPAD, WU), -1, np.int64)
    elog[:N_NODES, 0] = np.arange(N_NODES)
    elog[d_sorted, 1 + ranks] = row_d
    # node d sits at core d//NPC, block (d%NPC)//P, slot d%P
    elog_src = np.ascontiguousarray(
        elog.reshape(NCORES, BPC, P, WU).transpose(0, 2, 1, 3)
    ).reshape(NCORES, P, BPC * WU)
    srcp1 = np.where(elog_src >= 0, elog_src + 1, 0).astype(np.float32)

    return dict(
        row=row, col=col, WU=WU, elog_src=elog_src, srcp1=srcp1, inv=inv,
        own_k=own_k, own_b=own_b, own_p=own_p,
    )


def _preprocess_leaders(ppe, lead):
    """Leader-dependent preprocessing: leader layout + paired gather
    structure for launch C + paired final-gather structure for launch D."""
    row, col = ppe["row"], ppe["col"]
    leaders = np.unique(lead[:N_NODES])
    NLEAD = leaders.size
    NB2 = int((-(-NLEAD // NCORES) + P - 1) // P)  # blocks per core

    is_leader = np.zeros(NPAD, bool)
    is_leader[leaders] = True
    lid = np.full(NPAD, -1, np.int64)
    lid[leaders] = np.arange(NLEAD)

    emask = is_leader[col]
    fsrc = row[emask]
    fdst = col[emask]

    indeg = np.bincount(lid[fdst], minlength=NLEAD)[:, None]
    l2k, l2b, l2p, inv2 = _assign_nodes(leaders, indeg, NB2)

    fl = lid[fdst]
    ek = l2k[fl]
    edges_c = [(fsrc[ek == k], l2b[fl[ek == k]], l2p[fl[ek == k]])
               for k in range(NCORES)]
    gsc = _pack_paired(edges_c, NB2, 24)

    # per-dst-slot reciprocal counts (full in-degree of the leader there)
    cnt_full = np.bincount(col, minlength=NPAD).astype(np.float64)
    rc = np.zeros((NCORES, P, NB2), np.float32)
    for k in range(NCORES):
        nd = inv2[k]                                  # [NB2, P]
        c = np.where(nd >= 0, cnt_full[np.maximum(nd, 0)], 0.0)
        rc[k] = (1.0 / np.maximum(c, 1.0)).T

    # position of each leader in the global report table [NCORES*NB2*128]
    pos = np.full(NPAD, 0, np.int64)
    pos[leaders] = (l2k * NB2 + l2b) * P + l2p

    # launch D: per core, per own-node reference (pos-of-leader, blk, slot)
    edges_d = []
    own_nodes = ppe["inv"].reshape(NCORES, BPC, P)
    for k in range(NCORES):
        nn = own_nodes[k].reshape(-1)                 # [BPC*P] node or -1
        m = nn >= 0
        q = pos[lead[np.maximum(nn, 0)]][m]
        bb = (np.arange(NPC) // P)[m]
        pp_ = (np.arange(NPC) % P)[m]
        edges_d.append((q, bb, pp_))
    gsd = _pack_paired(edges_d, BPC, 16)

    return dict(gsc=gsc, gsd=gsd, NB2=NB2, NLEAD=NLEAD, leaders=leaders,
                inv2=inv2, rc=rc, pos=pos)


# ---------------------------------------------------------------------------
# launch A: logits
# ---------------------------------------------------------------------------

def _build_la():
    CH = 14                    # blocks per chunk (98 = 7*14)
    nc = bacc.Bacc("TRN2", target_bir_lowering=False, debug=False,
                   num_devices=NCORES)
    xf_d = nc.dram_tensor("xf", [P, BPC * H], dt.float32,
                          kind="ExternalInput")
    wrep_d = nc.dram_tensor("wrep", [P, H], dt.float32, kind="ExternalInput")
    blead_d = nc.dram_tensor("blead", [P, 1], dt.float32, kind="ExternalInput")
    logits_o = nc.dram_tensor("logits_o", [P, BPC], dt.float32,
                              kind="ExternalOutput")
    with tile.TileContext(nc) as tc:
        with (
            tc.tile_pool(name="const", bufs=1) as cp,
            tc.tile_pool(name="xt", bufs=3) as xp,
            tc.tile_pool(name="small", bufs=4) as sp,
        ):
            wrep_t = cp.tile([P, H], dt.float32)
            nc.sync.dma_start(wrep_t[:], wrep_d[:, :])
            blead_t = cp.tile([P, 1], dt.float32)
            nc.sync.dma_start(blead_t[:], blead_d[:, :])
            logits_sb = cp.tile([P, BPC], dt.float32)
            wrep_b = bass.AP(wrep_t.tensor, 0,
                             [wrep_t[:].ap[0], [0, CH], [1, H]])
            for c0 in range(0, BPC, CH):
                xt = xp.tile([P, CH, H], dt.float32, tag="xt")
                nc.sync.dma_start(xt[:], xf_d[:, c0 * H : (c0 + CH) * H]
                                  .rearrange("p (b h) -> p b h", h=H))
                tmp = xp.tile([P, CH, H], dt.float32, tag="ltmp")
                nc.vector.tensor_tensor(out=tmp[:], in0=xt[:], in1=wrep_b,
                                        op=mybir.AluOpType.mult)
                nc.vector.reduce_sum(out=logits_sb[:, c0 : c0 + CH],
                                     in_=tmp[:], axis=mybir.AxisListType.X)
            logits_out = sp.tile([P, BPC], dt.float32, tag="lgout")
            nc.vector.tensor_scalar_add(logits_out[:], logits_sb[:],
                                        blead_t[:, :1])
            nc.sync.dma_start(logits_o[:, :], logits_out[:])
    nc.compile()
    return nc


# ---------------------------------------------------------------------------
# launch B: leader election
# ---------------------------------------------------------------------------

def _build_lb(WU):
    SW = BPC * WU
    nc = bacc.Bacc("TRN2", target_bir_lowering=False, debug=False,
                   num_devices=NCORES)
    ep_d = nc.dram_tensor("epad", [P, SW], dt.float32, kind="ExternalInput")
    sp1_d = nc.dram_tensor("srcp1", [P, SW], dt.float32, kind="ExternalInput")
    lead_o = nc.dram_tensor("lead_o", [P, BPC], dt.float32,
                            kind="ExternalOutput")
    with tile.TileContext(nc) as tc:
        with tc.tile_pool(name="sb", bufs=1) as sb:
            ep = sb.tile([P, BPC, WU], dt.float32)
            nc.sync.dma_start(ep[:], ep_d[:, :].rearrange("p (b w) -> p b w",
                                                          w=WU))
            sp1 = sb.tile([P, BPC, WU], dt.float32)
            nc.sync.dma_start(sp1[:], sp1_d[:, :].rearrange("p (b w) -> p b w",
                                                            w=WU))
            sm = sb.tile([P, BPC], dt.float32)
            nc.vector.reduce_max(out=sm[:], in_=ep[:], axis=mybir.AxisListType.X)
            mask = sb.tile([P, BPC, WU], dt.float32)
            sm_b = bass.AP(sm.tensor, 0, [sm[:].ap[0], [1, BPC], [0, WU]])
            nc.vector.tensor_tensor(out=mask[:], in0=ep[:], in1=sm_b,
                                    op=mybir.AluOpType.is_equal)
            cand = sb.tile([P, BPC, WU], dt.float32)
            nc.vector.tensor_tensor(out=cand[:], in0=mask[:], in1=sp1[:],
                                    op=mybir.AluOpType.mult)
            lp1 = sb.tile([P, BPC], dt.float32)
            nc.vector.reduce_max(out=lp1[:], in_=cand[:],
                                 axis=mybir.AxisListType.X)
            leadf = sb.tile([P, BPC], dt.float32)
            nc.vector.tensor_scalar(
                out=leadf[:], in0=lp1[:], scalar1=-1.0, scalar2=0.0,
                op0=mybir.AluOpType.add, op1=mybir.AluOpType.max,
            )
            nc.sync.dma_start(lead_o[:, :], leadf[:])
    nc.compile()
    return nc


# ---------------------------------------------------------------------------
# launch C: filtered segment mean + MLP -> leader reports (bf16)
# ---------------------------------------------------------------------------

def _build_lc(gs, NB2, rc_nb):
    occs = gs["occs"]
    NOCC = gs["NOCC"]
    NCOL = gs["NCOL"]
    NROWS = gs["NROWS"]
    cw = gs["cw"]
    NI16 = NCOL * 8

    nc = bacc.Bacc("TRN2", target_bir_lowering=False, debug=False,
                   num_devices=NCORES)
    xtab_d = nc.dram_tensor("xtab2", [NROWS, 2 * H], dt.bfloat16,
                            kind="ExternalInput")
    idx_d = nc.dram_tensor("idx16", [P, NI16], dt.int16, kind="ExternalInput")
    am_d = nc.dram_tensor("amat", [P, NOCC * 2 * P], dt.float8e4,
                          kind="ExternalInput")
    rc_d = nc.dram_tensor("rc2", [P, rc_nb * P], dt.float32,
                          kind="ExternalInput")
    w1_d = nc.dram_tensor("w1", [H, H], dt.bfloat16, kind="ExternalInput")
    b1_d = nc.dram_tensor("b1", [P, 1], dt.float32, kind="ExternalInput")
    w2_d = nc.dram_tensor("w2", [H, C], dt.bfloat16, kind="ExternalInput")
    b2_d = nc.dram_tensor("b2c", [C, 1], dt.float32, kind="ExternalInput")
    rep_o = nc.dram_tensor("rep_o", [P, NB2 * C], dt.bfloat16,
                           kind="ExternalOutput")

    STAGE_B = 10

    with tile.TileContext(nc) as tc:
        with (
            tc.tile_pool(name="const", bufs=1) as cp,
            tc.tile_pool(name="g", bufs=7) as gp,
            tc.tile_pool(name="small", bufs=4) as sp,
            tc.tile_pool(name="stage", bufs=2) as stp,
            tc.tile_pool(name="sums_ps", bufs=5, space="PSUM") as sums_pp,
            tc.tile_pool(name="mlp_ps", bufs=3, space="PSUM") as mlp_pp,
        ):
            idx16_t = cp.tile([P, NI16], dt.int16)
            nc.sync.dma_start(idx16_t[:], idx_d[:, :])

            win_tiles = {}
            win_range = gs["win_range"]

            def ensure_window(w):
                if w in win_tiles:
                    return win_tiles[w]
                c0, c1 = win_range[w]
                ncw = c1 - c0
                G = gp.tile([P, cw, 2 * H], dt.bfloat16, tag="g")
                nc.gpsimd.dma_gather(
                    out_ap=G[:, :ncw, :],
                    in_ap=xtab_d[:, :],
                    idxs_ap=idx16_t[:, c0 * 8 : (c0 + ncw) * 8],
                    num_idxs=ncw * 128,
                    num_idxs_reg=ncw * 128,
                    elem_size=2 * H,
                    single_packet=False,
                )
                win_tiles[w] = (G, c0)
                return win_tiles[w]

            # issue first gathers before the remaining const loads so the
            # gather sem lane isn't transitively tied to later DMAs
            for w in range(min(2, len(win_range))):
                ensure_window(w)

            am_t = cp.tile([P, NOCC, 2, P], dt.float8e4)
            AC = (NOCC + 3) // 4
            for a0 in range(0, NOCC, AC):
                a1 = min(a0 + AC, NOCC)
                nc.sync.dma_start(
                    am_t[:, a0:a1],
                    am_d[:, a0 * 2 * P : a1 * 2 * P].rearrange(
                        "p (o t d) -> p o t d", t=2, d=P))
            rc_t = cp.tile([P, rc_nb, P], dt.float32)
            nc.sync.dma_start(rc_t[:], rc_d[:, :].rearrange(
                "p (b d) -> p b d", d=P))
            w1_t = cp.tile([H, H], dt.bfloat16)
            nc.sync.dma_start(w1_t[:], w1_d[:, :])
            b1_t = cp.tile([P, 1], dt.float32)
            nc.sync.dma_start(b1_t[:], b1_d[:, :])
            w2_t = cp.tile([H, C], dt.bfloat16)
            nc.sync.dma_start(w2_t[:], w2_d[:, :])
            b2_t = cp.tile([C, 1], dt.float32)
            nc.sync.dma_start(b2_t[:], b2_d[:, :])

            stage_t = None
            nblk = 0
            for b in range(NB2):
                ol = gs["occ_of_block"][b]
                # sums accumulated TRANSPOSED: [feature, dst]
                sums_ps = sums_pp.tile([P, P], dt.float32, space="PSUM",
                                       tag="sums")
                nref = len(ol)
                for i, j in enumerate(ol):
                    c, _b, lo, hi = occs[j]
                    w = int(gs["win_of_col"][c])
                    G, c0 = ensure_window(w)
                    for h in range(2):
                        nc.tensor.matmul(
                            out=sums_ps[:],
                            lhsT=G[:, c - c0, h * H : (h + 1) * H],
                            rhs=am_t[:, j, h, :],
                            start=(i == 0 and h == 0),
                            stop=(i == nref - 1 and h == 1))

                meanT_sb = sp.tile([P, P], dt.bfloat16, tag="meanT")
                nc.vector.tensor_tensor(out=meanT_sb[:], in0=sums_ps[:],
                                        in1=rc_t[:, b, :],
                                        op=mybir.AluOpType.mult)
                hpre_ps = mlp_pp.tile([P, P], dt.float32, space="PSUM",
                                      tag="mlp")
                nc.tensor.matmul(out=hpre_ps[:], lhsT=w1_t[:], rhs=meanT_sb[:],
                                 start=True, stop=True)
                hT_sb = sp.tile([P, P], dt.bfloat16, tag="hT")
                nc.scalar.activation(hT_sb[:], hpre_ps[:],
                                     mybir.ActivationFunctionType.Gelu,
                                     bias=b1_t[:, :1])
                rep_ps = mlp_pp.tile([P, P], dt.float32, space="PSUM",
                                     tag="mlp")
                nc.tensor.matmul(out=rep_ps[:], lhsT=w2_t[:], rhs=hT_sb[:],
                                 start=True, stop=True)

                sj = b % STAGE_B
                if sj == 0:
                    nblk = min(STAGE_B, NB2 - b)
                    stage_t = stp.tile([P, STAGE_B * C], dt.bfloat16,
                                       tag="stage")
                # rows land transposed: partition = report channel, free = dst
                nc.vector.tensor_scalar_add(
                    stage_t[:, sj * C : (sj + 1) * C], rep_ps[:],
                    b2_t[:, :1])
                if sj == nblk - 1:
                    b0 = b - sj
                    nc.sync.dma_start(
                        rep_o[:, b0 * C : (b0 + sj + 1) * C],
                        stage_t[:, : (sj + 1) * C],
                    )
    nc.compile()
    return nc


# ---------------------------------------------------------------------------
# launch D: final gather out[n] = reports[leader[n]]
# ---------------------------------------------------------------------------

def _build_ld(gs):
    occs = gs["occs"]
    NOCC = gs["NOCC"]
    NCOL = gs["NCOL"]
    NROWS = gs["NROWS"]
    cw = gs["cw"]
    NI16 = NCOL * 8
    STAGE_B = 14

    nc = bacc.Bacc("TRN2", target_bir_lowering=False, debug=False,
                   num_devices=NCORES)
    rep_d = nc.dram_tensor("reptab2", [NROWS, 2 * C], dt.bfloat16,
                           kind="ExternalInput")
    idx_d = nc.dram_tensor("ldidx16", [P, NI16], dt.int16,
                           kind="ExternalInput")
    em_d = nc.dram_tensor("emat", [P, NOCC * 2 * P], dt.float8e4,
                          kind="ExternalInput")
    out_o = nc.dram_tensor("gath_o", [P, BPC * C], dt.float32,
                           kind="ExternalOutput")
    with tile.TileContext(nc) as tc:
        with (
            tc.tile_pool(name="sb", bufs=1) as sb,
            tc.tile_pool(name="g", bufs=4) as gp,
            tc.tile_pool(name="stage", bufs=3) as stp,
            tc.tile_pool(name="ps", bufs=6, space="PSUM") as pp,
        ):
            idx_t = sb.tile([P, NI16], dt.int16)
            nc.sync.dma_start(idx_t[:], idx_d[:, :])

            win_tiles = {}
            win_range = gs["win_range"]

            def ensure_window(w):
                if w in win_tiles:
                    return win_tiles[w]
                c0, c1 = win_range[w]
                ncw = c1 - c0
                G = gp.tile([P, cw, 2 * C], dt.bfloat16, tag="g")
                nc.gpsimd.dma_gather(
                    out_ap=G[:, :ncw, :],
                    in_ap=rep_d[:, :],
                    idxs_ap=idx_t[:, c0 * 8 : (c0 + ncw) * 8],
                    num_idxs=ncw * 128,
                    num_idxs_reg=ncw * 128,
                    elem_size=2 * C,
                    single_packet=False,
                )
                win_tiles[w] = (G, c0)
                return win_tiles[w]

            for w in range(min(2, len(win_range))):
                ensure_window(w)

            em_t = sb.tile([P, NOCC, 2, P], dt.float8e4)
            AC = (NOCC + 3) // 4
            for a0 in range(0, NOCC, AC):
                a1 = min(a0 + AC, NOCC)
                nc.sync.dma_start(
                    em_t[:, a0:a1],
                    em_d[:, a0 * 2 * P : a1 * 2 * P].rearrange(
                        "p (o t d) -> p o t d", t=2, d=P))

            stage_t = None
            nblk = 0
            for bi, b in enumerate(gs["block_order"]):
                ol = gs["occ_of_block"][b]
                out_ps = pp.tile([P, C], dt.float32, space="PSUM", tag="o")
                nref = len(ol)
                for i, j in enumerate(ol):
                    c, _b, lo, hi = occs[j]
                    w = int(gs["win_of_col"][c])
                    G, c0 = ensure_window(w)
                    for h in range(2):
                        nc.tensor.matmul(
                            out=out_ps[:], lhsT=em_t[:, j, h, :],
                            rhs=G[:, c - c0, h * C : (h + 1) * C],
                            start=(i == 0 and h == 0),
                            stop=(i == nref - 1 and h == 1))
                sj = bi % STAGE_B
                if sj == 0:
                    nblk = min(STAGE_B, BPC - bi)
                    stage_t = stp.tile([P, STAGE_B * C], dt.float32,
                                       tag="stage")
                nc.scalar.activation(stage_t[:, sj * C : (sj + 1) * C],
                                     out_ps[:],
                                     mybir.ActivationFunctionType.Copy)
                if sj == nblk - 1:
                    b0 = bi - sj
                    nc.sync.dma_start(
                        out_o[:, b0 * C : (b0 + sj + 1) * C],
                        stage_t[:, : (sj + 1) * C],
                    )
    nc.compile()
    return nc


# ---------------------------------------------------------------------------

_CACHE = {}


def _get(key, fn):
    if key not in _CACHE:
        _CACHE[key] = fn()
    return _CACHE[key]


def kernel(x, edge_index, w_lead, b_lead, w1, b1, w2, b2):
    x = np.asarray(x, np.float32)
    N = x.shape[0]
    assert N == N_NODES and x.shape[1] == H

    ekey = hashlib.md5(np.asarray(edge_index).tobytes()).hexdigest()
    ppe = _get(("ppe", ekey), lambda: _preprocess_edges(edge_index))

    xpad = np.zeros((NPAD, H), np.float32)
    xpad[:N] = x
    wrep = np.tile(np.asarray(w_lead, np.float32)[None, :], (P, 1))
    blead = np.full((P, 1), np.float32(b_lead), np.float32)

    # ---- launch A: logits ---------------------------------------------------
    nca = _get("la", _build_la)
    in_a = [{
        "xf": np.ascontiguousarray(
            xpad[k * NPC : (k + 1) * NPC].reshape(BPC, P, H)
            .transpose(1, 0, 2).reshape(P, BPC * H)),
        "wrep": wrep,
        "blead": blead,
    } for k in range(NCORES)]
    ra = run_bass_kernel_spmd(nca, in_a, core_ids=CORES)

    logits_full = np.zeros(NPAD, np.float32)
    for k in range(NCORES):
        lg = ra.results[k]["logits_o"]            # [P, BPC]
        logits_full[k * NPC : (k + 1) * NPC] = lg.T.reshape(-1)
    logits_full[N:] = NEG

    # ---- launch B: election -------------------------------------------------
    ncb = _get(("lb", ekey), lambda: _build_lb(ppe["WU"]))
    es = ppe["elog_src"]
    in_b = [{
        "epad": np.ascontiguousarray(
            np.where(es[k] >= 0, logits_full[np.maximum(es[k], 0)],
                     NEG).astype(np.float32)),
        "srcp1": ppe["srcp1"][k],
    } for k in range(NCORES)]
    rb = run_bass_kernel_spmd(ncb, in_b, core_ids=CORES)

    lead = np.zeros(NPAD, np.int64)
    for k in range(NCORES):
        lf = rb.results[k]["lead_o"]              # [P, BPC] f32 node ids
        lead[k * NPC : (k + 1) * NPC] = lf.T.reshape(-1).astype(np.int64)

    # ---- leader-dependent preprocessing ------------------------------------
    lkey = hashlib.md5(lead.tobytes()).hexdigest()
    ppl = _get(("ppl", ekey, lkey), lambda: _preprocess_leaders(ppe, lead))
    gsc, gsd, NB2 = ppl["gsc"], ppl["gsd"], ppl["NB2"]

    # ---- launch C: filtered mean + MLP (paired gather) ---------------------
    xbf = xpad.astype(bf16)
    w1f = np.ascontiguousarray(np.asarray(w1, np.float32).astype(bf16))
    b1c = np.ascontiguousarray(np.asarray(b1, np.float32).reshape(H, 1))
    w2f = np.ascontiguousarray(np.asarray(w2, np.float32).astype(bf16))
    b2c = np.ascontiguousarray(np.asarray(b2, np.float32).reshape(C, 1))

    structure_key = hashlib.md5(
        np.asarray(gsc["occs"], np.int64).tobytes()
        + np.int64(NB2).tobytes() + np.int64(gsc["NROWS"]).tobytes()
    ).hexdigest()
    ncc = _get(("lc", structure_key), lambda: _build_lc(gsc, NB2, NB2))
    in_c = []
    for k in range(NCORES):
        pu, pv = gsc["pairs"][k]
        xtab2 = np.zeros((gsc["NROWS"], 2 * H), bf16)
        xtab2[: len(pu), 0:H] = xbf[pu]
        xtab2[: len(pu), H : 2 * H] = xbf[pv]
        rcf = ppl["rc"][k].T.reshape(-1)          # [NB2*P], [b*128+d]
        in_c.append({
            "xtab2": xtab2,
            "idx16": gsc["idx16"][k],
            "amat": gsc["amat"][k],
            "rc2": np.ascontiguousarray(
                np.broadcast_to(rcf[None, :], (P, NB2 * P))),
            "w1": w1f,
            "b1": b1c,
            "w2": w2f,
            "b2c": b2c,
        })
    rc_ = run_bass_kernel_spmd(ncc, in_c, core_ids=CORES)

    # ---- assemble leader report table --------------------------------------
    tab_rows = NCORES * NB2 * P
    reptab = np.zeros((tab_rows, C), bf16)
    for k in range(NCORES):
        rp = rc_.results[k]["rep_o"]              # [C, NB2*P] bf16 transposed
        reptab[k * NB2 * P : (k + 1) * NB2 * P] = rp.T

    # ---- launch D: final gather (paired) -----------------------------------
    structure_key_d = hashlib.md5(
        np.asarray(gsd["occs"], np.int64).tobytes()
        + np.int64(gsd["NROWS"]).tobytes()).hexdigest()
    ncd = _get(("ld", structure_key_d), lambda: _build_ld(gsd))
    in_d = []
    for k in range(NCORES):
        qu, qv = gsd["pairs"][k]
        rt2 = np.zeros((gsd["NROWS"], 2 * C), bf16)
        rt2[: len(qu), 0:C] = reptab[qu]
        rt2[: len(qu), C : 2 * C] = reptab[qv]
        in_d.append({
            "reptab2": rt2,
            "ldidx16": gsd["idx16"][k],
            "emat": gsd["amat"][k],
        })
    rd = run_bass_kernel_spmd(ncd, in_d, core_ids=CORES)

    out = np.zeros((N, C), np.float32)
    bo = np.asarray(gsd["block_order"], np.int64)
    for k in range(NCORES):
        g = rd.results[k]["gath_o"].reshape(P, BPC, C)
        g_true = np.empty_like(g)
        g_true[:, bo, :] = g
        node_rows = g_true.transpose(1, 0, 2).reshape(NPC, C)
        n0 = k * NPC
        n1 = min((k + 1) * NPC, N)
        out[n0:n1] = node_rows[: n1 - n0]
    return out


# revision 43
# speedup vs baseline: 1.1820x; 1.1820x over previous
"""Trainium2 Bass kernel for nn_DecentralizedCoordinator (GNN message passing).

Strategy (8 NeuronCores, SPMD, 4 launches). The SWDGE gather descriptor
rate (~7-8 ns/row on the GpSimd Q7, regardless of row size) is the machine
bottleneck for message passing, so the design minimizes gathered rows:

- L-A  logits = x @ w_lead + b (nodes sharded by id, one block-major DMA +
  batched DVE mult/reduce per core).
- host gathers logits into a per-dst padded layout (pure index routing).
- L-B  leader election per dst (reduce_max / is_equal / mult(src+1) /
  reduce_max; exact reference tie-break semantics) -> leader id per node.
- host: only ~31% of nodes are ever somebody's leader and only their
  reports are read. Distinct leaders are re-balanced across cores/blocks;
  only edges into leader dsts (~30k/core instead of ~100k/core) are kept.
  Referenced source nodes are greedily MATCHED INTO PAIRS within dst
  blocks; one 512B dma_gather descriptor fetches a pair-row [x_u | x_v]
  from a per-core compacted table (single int16 subtable), cutting
  descriptors another ~43%. Per-(column,block) occurrence fp8 scatter
  matrices (one per pair half, host-precomputed, exact small ints) route
  each half's edges to dst slots with multiplicity.
- L-C  segment mean + report MLP for leader nodes only: scatter matmuls
  (lhsT = gathered bf16 half, rhs = fp8 matrix) accumulate sums
  TRANSPOSED [feature, dst] in PSUM; per-dst 1/max(cnt,1) (host index
  metadata) applied on DVE; w1 -> gelu(+b1) -> w2 (+b2 on DVE) without any
  on-chip transpose; reports written bf16 transposed.
- host assembles the global leader-report table + per-node positions.
- L-D  out[n] = reports[leader[n]]: same pair-matched gather (position
  pairs co-referenced by an output block share a 512B descriptor) + fp8
  one-hot expansion matmuls into f32 PSUM (cast for free via scalar copy).

Host only shards/reshapes/gathers-by-index between launches; all
arithmetic on values happens on device.  1093us -> ~351us measured.
"""
import os
import sys
import hashlib

import numpy as np
import ml_dtypes

sys.path.insert(0, "/opt/trn_rl_repo")

import concourse.bass as bass
import concourse.tile as tile
from concourse import bacc, mybir
from concourse.bass_utils import run_bass_kernel_spmd
from concourse.masks import make_identity

dt = mybir.dt
bf16 = ml_dtypes.bfloat16

P = 128
NCORES = 8
BPC = 98                 # dst blocks per core (all-nodes layout)
NPC = BPC * P            # 12544 nodes per core
NPAD = NCORES * NPC      # 100352 padded node count
NSUB = 4
SUB = NPAD // NSUB       # 25088 rows per gather subtable
H = 128
C = 128
CW = 32                  # gather-window width (columns of 128 rows)
NEG = -3.0e38
N_NODES = 100000

CORES = list(range(NCORES))


def _wrap_idx16(local_idx):
    """[NI] int array -> [128, NI//16] int16 (16-wrap, replicated x8)."""
    ni = len(local_idx)
    assert ni % 16 == 0
    w = np.asarray(local_idx, np.int16).reshape(ni // 16, 16).T  # [16, NI/16]
    return np.tile(w, (8, 1)).copy()


def _assign_nodes(nodes, indeg4, nblk):
    """Balanced node -> (core, block, slot) over NCORES x nblk x P slots.

    nodes: [M] node ids to place; indeg4: [M, NSUB] per-subtable in-degree.
    Equalizes per-(block, sub) in-degree across cores (vector-aware greedy).
    Returns node2kbp {node: (k, b, p)} arrays and inv [NCORES, nblk, P].
    """
    M = len(nodes)
    assert M <= NCORES * nblk * P
    indeg = indeg4.sum(axis=1)
    order = np.argsort(-indeg, kind="stable")
    n2k = np.zeros(M, np.int64)
    n2b = np.zeros(M, np.int64)
    n2p = np.zeros(M, np.int64)
    inv = np.full((NCORES, nblk, P), -1, np.int64)
    BIG = 1 << 40
    per_blk = NCORES * P
    for b in range(nblk):
        sl = order[b * per_blk : (b + 1) * per_blk]
        loads = np.zeros((NCORES, indeg4.shape[1]), np.float64)
        caps = np.full(NCORES, P, np.int64)
        slots = np.zeros(NCORES, np.int64)
        for j in sl:
            v = indeg4[j]
            cost = ((loads + v) ** 2).sum(axis=1)
            cost[caps == 0] = BIG
            k = int(np.argmin(cost))
            loads[k] += v
            caps[k] -= 1
            p = int(slots[k])
            slots[k] += 1
            n2k[j] = k
            n2b[j] = b
            n2p[j] = p
            inv[k, b, p] = nodes[j]
    return n2k, n2b, n2p, inv


def _pack_paired(edges_per_core, nblk, cw, R=2, whole_blocks=False):
    """Paired gather packing. edges_per_core[k] = (src, blk, dstp) per-edge
    arrays (dst owned by core k). Nodes referenced by a core are greedily
    matched into pairs within blocks; a 512B descriptor fetches one pair-row
    [x_u | x_v]; a pair occupies one slot per block it has edges into, and
    per-occurrence fp8 scatter matrices (one per half) route each half's
    edges to their dst slots (with multiplicity).

    Returns shared structure (occs/windows, max over cores) + per-core
    (pairs, idx16, amat fp8, nrows).
    """
    # --- per-core grouping (R nodes per 256B*R descriptor row) ---------------
    pairs_uv = []        # per core: list of R arrays (group member r)
    hit_kb = []          # per core: (hb, hp, pid) sorted by (b, pid)
    for k in range(NCORES):
        src, blk, dstp = edges_per_core[k]
        key = src * nblk + blk
        uk = np.unique(key)
        us, ub = uk // nblk, uk % nblk
        order = np.argsort(ub, kind="stable")
        us_s, ub_s = us[order], ub[order]
        bnd = np.searchsorted(ub_s, np.arange(nblk + 1))
        matched = np.zeros(NPAD, bool)
        parts = [[] for _ in range(R)]
        for b in range(nblk):
            nodes = us_s[bnd[b] : bnd[b + 1]]
            nodes = nodes[~matched[nodes]]
            m = (len(nodes) // R) * R
            for r in range(R):
                parts[r].append(nodes[r:m:R])
            matched[nodes[:m]] = True
        alln = np.unique(us)
        singles = alln[~matched[alln]]
        if len(singles) % R:
            pad = R - len(singles) % R
            singles = np.concatenate([singles, np.repeat(singles[-1:], pad)])
        for r in range(R):
            parts[r].append(singles[r::R])
        grp = [np.concatenate(p).astype(np.int64) for p in parts]
        pairs_uv.append(grp)
        pid = np.full(NPAD, -1, np.int64)
        for r in range(R):
            pid[grp[r]] = np.arange(len(grp[0]))
        # distinct (group, block) hits, sorted by (b, pid)
        hkey = ub * (NPAD + 1) + pid[us]
        hk = np.unique(hkey)
        hb, hp = hk // (NPAD + 1), hk % (NPAD + 1)
        hit_kb.append((hb, hp, pid))

    # --- shared slot structure ----------------------------------------------
    cnt_kb = np.zeros((NCORES, nblk), np.int64)
    for k in range(NCORES):
        hb = hit_kb[k][0]
        cnt_kb[k] = np.bincount(hb, minlength=nblk)
    m_b = cnt_kb.max(axis=0)
    occs = []
    occ_of_block = [[] for _ in range(nblk)]
    occ_idx = {}
    if whole_blocks:
        # FFD bin-pack: each block's slots sit whole inside one column
        assert m_b.max() <= 128
        col_of_b = np.zeros(nblk, np.int64)
        lo_of_b = np.zeros(nblk, np.int64)
        space = []
        for b in np.argsort(-m_b):
            sz = int(m_b[b])
            if sz == 0:
                continue
            for c in range(len(space)):
                if space[c] >= sz:
                    lo_of_b[b] = 128 - space[c]
                    col_of_b[b] = c
                    space[c] -= sz
                    break
            else:
                col_of_b[b] = len(space)
                lo_of_b[b] = 0
                space.append(128 - sz)
        NCOL = len(space)
        slot_base = col_of_b * 128 + lo_of_b
        order_b = sorted(range(nblk),
                         key=lambda b: (int(col_of_b[b]), int(lo_of_b[b])))
        for b in order_b:
            if m_b[b] == 0:
                continue
            c, lo = int(col_of_b[b]), int(lo_of_b[b])
            occ_idx[(c, b)] = len(occs)
            occ_of_block[b].append(len(occs))
            occs.append((c, b, lo, lo + int(m_b[b])))
        block_order = order_b
    else:
        slot_off = np.concatenate([[0], np.cumsum(m_b)[:-1]])
        slot_base = slot_off
        total = int(m_b.sum())
        NCOL = (total + 127) // 128
        for b in range(nblk):
            s0 = int(slot_off[b])
            s1 = s0 + int(m_b[b])
            if s1 == s0:
                continue
            for c in range(s0 // 128, (s1 - 1) // 128 + 1):
                lo = max(s0, c * 128) - c * 128
                hi = min(s1, (c + 1) * 128) - c * 128
                occ_idx[(c, b)] = len(occs)
                occ_of_block[b].append(len(occs))
                occs.append((c, b, lo, hi))
        block_order = list(range(nblk))
    NOCC = len(occs)

    # windows of cw columns; split the final window in two so the tail
    # (compute that can only start after the last gather) stays small
    bounds = list(range(0, NCOL, cw)) + [NCOL]
    if len(bounds) >= 2 and bounds[-1] - bounds[-2] > cw // 2:
        bounds.insert(-1, (bounds[-2] + bounds[-1] + 1) // 2)
    win_range = {}
    win_of_col = np.zeros(NCOL, np.int64)
    for w in range(len(bounds) - 1):
        win_range[w] = (bounds[w], bounds[w + 1])
        win_of_col[bounds[w] : bounds[w + 1]] = w
    win_occ_range = {}
    for j, (c, b, lo, hi) in enumerate(occs):
        w = int(win_of_col[c])
        if w not in win_occ_range:
            win_occ_range[w] = [j, j + 1]
        else:
            win_occ_range[w][1] = j + 1

    # --- per-core idx + matrices --------------------------------------------
    NI = NCOL * 128
    idx16 = []
    amat = []
    nrows = []
    for k in range(NCORES):
        src, blk, dstp = edges_per_core[k]
        hb, hp, pid = hit_kb[k]
        grp = pairs_uv[k]
        nrows.append(len(grp[0]))
        # global slot of each hit: slot_base[b] + rank within block
        starts = np.searchsorted(hb, np.arange(nblk))
        rank = np.arange(len(hb)) - starts[hb]
        gslot = slot_base[hb] + rank
        idx_all = np.zeros(NI, np.int64)
        idx_all[gslot] = hp
        idx16.append(_wrap_idx16(idx_all))
        # per-edge: locate slot, group member (half), occurrence
        ep = pid[src]
        hkey = blk * (NPAD + 1) + ep
        hpos = np.searchsorted(hb * (NPAD + 1) + hp, hkey)
        es = gslot[hpos]
        half = np.full(len(src), -1, np.int64)
        for r in range(R - 1, -1, -1):
            half = np.where(grp[r][ep] == src, r, half)
        assert (half >= 0).all()
        ecol = es // 128
        srow = es % 128
        eocc = np.array([occ_idx[(int(c), int(b))]
                         for c, b in zip(ecol, blk)], np.int64)
        A = np.zeros((P, NOCC, R, P), np.float32)
        np.add.at(A, (srow, eocc, half, dstp), 1.0)
        amat.append(np.ascontiguousarray(
            A.reshape(P, NOCC * R * P)).astype(ml_dtypes.float8_e4m3))
    NROWS = max(nrows)

    return dict(
        occs=occs, occ_of_block=occ_of_block, NOCC=NOCC, NCOL=NCOL,
        win_range=win_range, win_occ_range=win_occ_range,
        idx16=idx16, amat=amat, pairs=pairs_uv, NROWS=NROWS, cw=cw, R=R,
        block_order=block_order, win_of_col=win_of_col,
    )


def _preprocess_edges(edge_index):
    """Edge-only preprocessing (independent of values): all-nodes layout,
    election layout."""
    row = np.asarray(edge_index[0], np.int64)
    col = np.asarray(edge_index[1], np.int64)

    # trivial node -> (core, block, slot) by id; last 352 slots empty
    ids = np.arange(NPAD, dtype=np.int64)
    own_k = ids // NPC
    own_b = (ids % NPC) // P
    own_p = ids % P
    inv = np.where(ids < N_NODES, ids, -1).reshape(NCORES, BPC, P)

    # extended-edge (leader) layout: uniform width WU per dst
    cnt = np.bincount(col, minlength=NPAD)
    WU = int(cnt.max()) + 1
    dorder = np.argsort(col, kind="stable")
    row_d = row[dorder]
    d_sorted = col[dorder]
    dst_starts = np.concatenate([[0], np.cumsum(cnt)])
    ranks = np.arange(len(col)) - np.repeat(dst_starts[:-1], cnt)
    elog = np.full((NPAD, WU), -1, np.int64)
    elog[:N_NODES, 0] = np.arange(N_NODES)
    elog[d_sorted, 1 + ranks] = row_d
    # node d sits at core d//NPC, block (d%NPC)//P, slot d%P
    elog_src = np.ascontiguousarray(
        elog.reshape(NCORES, BPC, P, WU).transpose(0, 2, 1, 3)
    ).reshape(NCORES, P, BPC * WU)
    srcp1 = np.where(elog_src >= 0, elog_src + 1, 0).astype(np.float32)

    return dict(
        row=row, col=col, WU=WU, elog_src=elog_src, srcp1=srcp1, inv=inv,
        own_k=own_k, own_b=own_b, own_p=own_p,
    )


def _preprocess_leaders(ppe, lead):
    """Leader-dependent preprocessing: leader layout + paired gather
    structure for launch C + paired final-gather structure for launch D."""
    row, col = ppe["row"], ppe["col"]
    leaders = np.unique(lead[:N_NODES])
    NLEAD = leaders.size
    NB2 = int((-(-NLEAD // NCORES) + P - 1) // P)  # blocks per core

    is_leader = np.zeros(NPAD, bool)
    is_leader[leaders] = True
    lid = np.full(NPAD, -1, np.int64)
    lid[leaders] = np.arange(NLEAD)

    emask = is_leader[col]
    fsrc = row[emask]
    fdst = col[emask]

    indeg = np.bincount(lid[fdst], minlength=NLEAD)[:, None]
    l2k, l2b, l2p, inv2 = _assign_nodes(leaders, indeg, NB2)

    fl = lid[fdst]
    ek = l2k[fl]
    edges_c = [(fsrc[ek == k], l2b[fl[ek == k]], l2p[fl[ek == k]])
               for k in range(NCORES)]
    gsc = _pack_paired(edges_c, NB2, 12, R=4)

    # per-dst-slot reciprocal counts (full in-degree of the leader there)
    cnt_full = np.bincount(col, minlength=NPAD).astype(np.float64)
    rc = np.zeros((NCORES, P, NB2), np.float32)
    for k in range(NCORES):
        nd = inv2[k]                                  # [NB2, P]
        c = np.where(nd >= 0, cnt_full[np.maximum(nd, 0)], 0.0)
        rc[k] = (1.0 / np.maximum(c, 1.0)).T

    # position of each leader in the global report table [NCORES*NB2*128]
    pos = np.full(NPAD, 0, np.int64)
    pos[leaders] = (l2k * NB2 + l2b) * P + l2p

    # launch D: per core, per own-node reference (pos-of-leader, blk, slot)
    edges_d = []
    own_nodes = ppe["inv"].reshape(NCORES, BPC, P)
    for k in range(NCORES):
        nn = own_nodes[k].reshape(-1)                 # [BPC*P] node or -1
        m = nn >= 0
        q = pos[lead[np.maximum(nn, 0)]][m]
        bb = (np.arange(NPC) // P)[m]
        pp_ = (np.arange(NPC) % P)[m]
        edges_d.append((q, bb, pp_))
    gsd = _pack_paired(edges_d, BPC, 16, R=3)

    return dict(gsc=gsc, gsd=gsd, NB2=NB2, NLEAD=NLEAD, leaders=leaders,
                inv2=inv2, rc=rc, pos=pos)


# ---------------------------------------------------------------------------
# launch A: logits
# ---------------------------------------------------------------------------

def _build_la():
    CH = 14                    # blocks per chunk (98 = 7*14)
    nc = bacc.Bacc("TRN2", target_bir_lowering=False, debug=False,
                   num_devices=NCORES)
    xf_d = nc.dram_tensor("xf", [P, BPC * H], dt.float32,
                          kind="ExternalInput")
    wrep_d = nc.dram_tensor("wrep", [P, H], dt.float32, kind="ExternalInput")
    blead_d = nc.dram_tensor("blead", [P, 1], dt.float32, kind="ExternalInput")
    logits_o = nc.dram_tensor("logits_o", [P, BPC], dt.float32,
                              kind="ExternalOutput")
    with tile.TileContext(nc) as tc:
        with (
            tc.tile_pool(name="const", bufs=1) as cp,
            tc.tile_pool(name="xt", bufs=3) as xp,
            tc.tile_pool(name="small", bufs=4) as sp,
        ):
            wrep_t = cp.tile([P, H], dt.float32)
            nc.sync.dma_start(wrep_t[:], wrep_d[:, :])
            blead_t = cp.tile([P, 1], dt.float32)
            nc.sync.dma_start(blead_t[:], blead_d[:, :])
            logits_sb = cp.tile([P, BPC], dt.float32)
            wrep_b = bass.AP(wrep_t.tensor, 0,
                             [wrep_t[:].ap[0], [0, CH], [1, H]])
            for ci, c0 in enumerate(range(0, BPC, CH)):
                xt = xp.tile([P, CH, H], dt.float32, tag="xt")
                eng = nc.sync if ci % 2 == 0 else nc.scalar
                eng.dma_start(xt[:], xf_d[:, c0 * H : (c0 + CH) * H]
                              .rearrange("p (b h) -> p b h", h=H))
                tmp = xp.tile([P, CH, H], dt.float32, tag="ltmp")
                nc.vector.tensor_tensor(out=tmp[:], in0=xt[:], in1=wrep_b,
                                        op=mybir.AluOpType.mult)
                nc.vector.reduce_sum(out=logits_sb[:, c0 : c0 + CH],
                                     in_=tmp[:], axis=mybir.AxisListType.X)
            logits_out = sp.tile([P, BPC], dt.float32, tag="lgout")
            nc.vector.tensor_scalar_add(logits_out[:], logits_sb[:],
                                        blead_t[:, :1])
            nc.sync.dma_start(logits_o[:, :], logits_out[:])
    nc.compile()
    return nc


# ---------------------------------------------------------------------------
# launch B: leader election
# ---------------------------------------------------------------------------

def _build_lb(WU):
    SW = BPC * WU
    nc = bacc.Bacc("TRN2", target_bir_lowering=False, debug=False,
                   num_devices=NCORES)
    ep_d = nc.dram_tensor("epad", [P, SW], dt.float32, kind="ExternalInput")
    sp1_d = nc.dram_tensor("srcp1", [P, SW], dt.float32, kind="ExternalInput")
    lead_o = nc.dram_tensor("lead_o", [P, BPC], dt.float32,
                            kind="ExternalOutput")
    with tile.TileContext(nc) as tc:
        with tc.tile_pool(name="sb", bufs=1) as sb:
            ep = sb.tile([P, BPC, WU], dt.float32)
            nc.sync.dma_start(ep[:], ep_d[:, :].rearrange("p (b w) -> p b w",
                                                          w=WU))
            sp1 = sb.tile([P, BPC, WU], dt.float32)
            nc.sync.dma_start(sp1[:], sp1_d[:, :].rearrange("p (b w) -> p b w",
                                                            w=WU))
            sm = sb.tile([P, BPC], dt.float32)
            nc.vector.reduce_max(out=sm[:], in_=ep[:], axis=mybir.AxisListType.X)
            mask = sb.tile([P, BPC, WU], dt.float32)
            sm_b = bass.AP(sm.tensor, 0, [sm[:].ap[0], [1, BPC], [0, WU]])
            nc.vector.tensor_tensor(out=mask[:], in0=ep[:], in1=sm_b,
                                    op=mybir.AluOpType.is_equal)
            cand = sb.tile([P, BPC, WU], dt.float32)
            nc.vector.tensor_tensor(out=cand[:], in0=mask[:], in1=sp1[:],
                                    op=mybir.AluOpType.mult)
            lp1 = sb.tile([P, BPC], dt.float32)
            nc.vector.reduce_max(out=lp1[:], in_=cand[:],
                                 axis=mybir.AxisListType.X)
            leadf = sb.tile([P, BPC], dt.float32)
            nc.vector.tensor_scalar(
                out=leadf[:], in0=lp1[:], scalar1=-1.0, scalar2=0.0,
                op0=mybir.AluOpType.add, op1=mybir.AluOpType.max,
            )
            nc.sync.dma_start(lead_o[:, :], leadf[:])
    nc.compile()
    return nc


# ---------------------------------------------------------------------------
# launch C: filtered segment mean + MLP -> leader reports (bf16)
# ---------------------------------------------------------------------------

def _build_lc(gs, NB2, rc_nb):
    occs = gs["occs"]
    NOCC = gs["NOCC"]
    NCOL = gs["NCOL"]
    NROWS = gs["NROWS"]
    cw = gs["cw"]
    R = gs["R"]
    NI16 = NCOL * 8

    nc = bacc.Bacc("TRN2", target_bir_lowering=False, debug=False,
                   num_devices=NCORES)
    xtab_d = nc.dram_tensor("xtab2", [NROWS, R * H], dt.bfloat16,
                            kind="ExternalInput")
    idx_d = nc.dram_tensor("idx16", [P, NI16], dt.int16, kind="ExternalInput")
    am_d = nc.dram_tensor("amat", [P, NOCC * R * P], dt.float8e4,
                          kind="ExternalInput")
    rc_d = nc.dram_tensor("rc2", [P, rc_nb * P], dt.float32,
                          kind="ExternalInput")
    w1_d = nc.dram_tensor("w1", [H, H], dt.bfloat16, kind="ExternalInput")
    b1_d = nc.dram_tensor("b1", [P, 1], dt.float32, kind="ExternalInput")
    w2_d = nc.dram_tensor("w2", [H, C], dt.bfloat16, kind="ExternalInput")
    b2_d = nc.dram_tensor("b2c", [C, 1], dt.float32, kind="ExternalInput")
    rep_o = nc.dram_tensor("rep_o", [P, NB2 * C], dt.bfloat16,
                           kind="ExternalOutput")

    STAGE_B = 10

    with tile.TileContext(nc) as tc:
        with (
            tc.tile_pool(name="const", bufs=1) as cp,
            tc.tile_pool(name="g", bufs=7) as gp,
            tc.tile_pool(name="small", bufs=4) as sp,
            tc.tile_pool(name="stage", bufs=2) as stp,
            tc.tile_pool(name="sums_ps", bufs=5, space="PSUM") as sums_pp,
            tc.tile_pool(name="mlp_ps", bufs=3, space="PSUM") as mlp_pp,
        ):
            idx16_t = cp.tile([P, NI16], dt.int16)
            nc.sync.dma_start(idx16_t[:], idx_d[:, :])

            win_tiles = {}
            win_range = gs["win_range"]

            def ensure_window(w):
                if w in win_tiles:
                    return win_tiles[w]
                c0, c1 = win_range[w]
                ncw = c1 - c0
                G = gp.tile([P, cw, R * H], dt.bfloat16, tag="g")
                nc.gpsimd.dma_gather(
                    out_ap=G[:, :ncw, :],
                    in_ap=xtab_d[:, :],
                    idxs_ap=idx16_t[:, c0 * 8 : (c0 + ncw) * 8],
                    num_idxs=ncw * 128,
                    num_idxs_reg=ncw * 128,
                    elem_size=R * H,
                    single_packet=False,
                )
                win_tiles[w] = (G, c0)
                return win_tiles[w]

            # issue first gathers before the remaining const loads so the
            # gather sem lane isn't transitively tied to later DMAs
            for w in range(min(2, len(win_range))):
                ensure_window(w)

            am_t = cp.tile([P, NOCC, R, P], dt.float8e4)
            AC = (NOCC + 3) // 4
            for a0 in range(0, NOCC, AC):
                a1 = min(a0 + AC, NOCC)
                nc.sync.dma_start(
                    am_t[:, a0:a1],
                    am_d[:, a0 * R * P : a1 * R * P].rearrange(
                        "p (o t d) -> p o t d", t=R, d=P))
            rc_t = cp.tile([P, rc_nb, P], dt.float32)
            nc.sync.dma_start(rc_t[:], rc_d[:, :].rearrange(
                "p (b d) -> p b d", d=P))
            w1_t = cp.tile([H, H], dt.bfloat16)
            nc.sync.dma_start(w1_t[:], w1_d[:, :])
            b1_t = cp.tile([P, 1], dt.float32)
            nc.sync.dma_start(b1_t[:], b1_d[:, :])
            w2_t = cp.tile([H, C], dt.bfloat16)
            nc.sync.dma_start(w2_t[:], w2_d[:, :])
            b2_t = cp.tile([C, 1], dt.float32)
            nc.sync.dma_start(b2_t[:], b2_d[:, :])

            stage_t = None
            nblk = 0
            for b in range(NB2):
                ol = gs["occ_of_block"][b]
                # sums accumulated TRANSPOSED: [feature, dst]
                sums_ps = sums_pp.tile([P, P], dt.float32, space="PSUM",
                                       tag="sums")
                nref = len(ol)
                for i, j in enumerate(ol):
                    c, _b, lo, hi = occs[j]
                    w = int(gs["win_of_col"][c])
                    G, c0 = ensure_window(w)
                    for h in range(R):
                        nc.tensor.matmul(
                            out=sums_ps[:],
                            lhsT=G[:, c - c0, h * H : (h + 1) * H],
                            rhs=am_t[:, j, h, :],
                            start=(i == 0 and h == 0),
                            stop=(i == nref - 1 and h == R - 1))

                meanT_sb = sp.tile([P, P], dt.bfloat16, tag="meanT")
                nc.vector.tensor_tensor(out=meanT_sb[:], in0=sums_ps[:],
                                        in1=rc_t[:, b, :],
                                        op=mybir.AluOpType.mult)
                hpre_ps = mlp_pp.tile([P, P], dt.float32, space="PSUM",
                                      tag="mlp")
                nc.tensor.matmul(out=hpre_ps[:], lhsT=w1_t[:], rhs=meanT_sb[:],
                                 start=True, stop=True)
                hT_sb = sp.tile([P, P], dt.bfloat16, tag="hT")
                nc.scalar.activation(hT_sb[:], hpre_ps[:],
                                     mybir.ActivationFunctionType.Gelu,
                                     bias=b1_t[:, :1])
                rep_ps = mlp_pp.tile([P, P], dt.float32, space="PSUM",
                                     tag="mlp")
                nc.tensor.matmul(out=rep_ps[:], lhsT=w2_t[:], rhs=hT_sb[:],
                                 start=True, stop=True)

                sj = b % STAGE_B
                if sj == 0:
                    nblk = min(STAGE_B, NB2 - b)
                    stage_t = stp.tile([P, STAGE_B * C], dt.bfloat16,
                                       tag="stage")
                # rows land transposed: partition = report channel, free = dst
                nc.vector.tensor_scalar_add(
                    stage_t[:, sj * C : (sj + 1) * C], rep_ps[:],
                    b2_t[:, :1])
                if sj == nblk - 1:
                    b0 = b - sj
                    nc.sync.dma_start(
                        rep_o[:, b0 * C : (b0 + sj + 1) * C],
                        stage_t[:, : (sj + 1) * C],
                    )
    nc.compile()
    return nc


# ---------------------------------------------------------------------------
# launch D: final gather out[n] = reports[leader[n]]
# ---------------------------------------------------------------------------

def _build_ld(gs):
    occs = gs["occs"]
    NOCC = gs["NOCC"]
    NCOL = gs["NCOL"]
    NROWS = gs["NROWS"]
    cw = gs["cw"]
    R = gs["R"]
    NI16 = NCOL * 8
    STAGE_B = 14

    nc = bacc.Bacc("TRN2", target_bir_lowering=False, debug=False,
                   num_devices=NCORES)
    rep_d = nc.dram_tensor("reptab2", [NROWS, R * C], dt.bfloat16,
                           kind="ExternalInput")
    idx_d = nc.dram_tensor("ldidx16", [P, NI16], dt.int16,
                           kind="ExternalInput")
    em_d = nc.dram_tensor("emat", [P, NOCC * R * P], dt.float8e4,
                          kind="ExternalInput")
    out_o = nc.dram_tensor("gath_o", [P, BPC * C], dt.float32,
                           kind="ExternalOutput")
    with tile.TileContext(nc) as tc:
        with (
            tc.tile_pool(name="sb", bufs=1) as sb,
            tc.tile_pool(name="g", bufs=5) as gp,
            tc.tile_pool(name="stage", bufs=3) as stp,
            tc.tile_pool(name="ps", bufs=6, space="PSUM") as pp,
        ):
            idx_t = sb.tile([P, NI16], dt.int16)
            nc.sync.dma_start(idx_t[:], idx_d[:, :])

            win_tiles = {}
            win_range = gs["win_range"]

            def ensure_window(w):
                if w in win_tiles:
                    return win_tiles[w]
                c0, c1 = win_range[w]
                ncw = c1 - c0
                G = gp.tile([P, cw, R * C], dt.bfloat16, tag="g")
                nc.gpsimd.dma_gather(
                    out_ap=G[:, :ncw, :],
                    in_ap=rep_d[:, :],
                    idxs_ap=idx_t[:, c0 * 8 : (c0 + ncw) * 8],
                    num_idxs=ncw * 128,
                    num_idxs_reg=ncw * 128,
                    elem_size=R * C,
                    single_packet=False,
                )
                win_tiles[w] = (G, c0)
                return win_tiles[w]

            for w in range(min(2, len(win_range))):
                ensure_window(w)

            em_t = sb.tile([P, NOCC, R, P], dt.float8e4)
            AC = (NOCC + 3) // 4
            for a0 in range(0, NOCC, AC):
                a1 = min(a0 + AC, NOCC)
                nc.sync.dma_start(
                    em_t[:, a0:a1],
                    em_d[:, a0 * R * P : a1 * R * P].rearrange(
                        "p (o t d) -> p o t d", t=R, d=P))

            stage_t = None
            nblk = 0
            for bi, b in enumerate(gs["block_order"]):
                ol = gs["occ_of_block"][b]
                out_ps = pp.tile([P, C], dt.float32, space="PSUM", tag="o")
                nref = len(ol)
                for i, j in enumerate(ol):
                    c, _b, lo, hi = occs[j]
                    w = int(gs["win_of_col"][c])
                    G, c0 = ensure_window(w)
                    for h in range(R):
                        nc.tensor.matmul(
                            out=out_ps[:], lhsT=em_t[:, j, h, :],
                            rhs=G[:, c - c0, h * C : (h + 1) * C],
                            start=(i == 0 and h == 0),
                            stop=(i == nref - 1 and h == R - 1))
                sj = bi % STAGE_B
                if sj == 0:
                    nblk = min(STAGE_B, BPC - bi)
                    stage_t = stp.tile([P, STAGE_B * C], dt.float32,
                                       tag="stage")
                nc.scalar.activation(stage_t[:, sj * C : (sj + 1) * C],
                                     out_ps[:],
                                     mybir.ActivationFunctionType.Copy)
                if sj == nblk - 1:
                    b0 = bi - sj
                    nc.sync.dma_start(
                        out_o[:, b0 * C : (b0 + sj + 1) * C],
                        stage_t[:, : (sj + 1) * C],
                    )
    nc.compile()
    return nc


# ---------------------------------------------------------------------------

_CACHE = {}


def _get(key, fn):
    if key not in _CACHE:
        _CACHE[key] = fn()
    return _CACHE[key]


def kernel(x, edge_index, w_lead, b_lead, w1, b1, w2, b2):
    x = np.asarray(x, np.float32)
    N = x.shape[0]
    assert N == N_NODES and x.shape[1] == H

    ekey = hashlib.md5(np.asarray(edge_index).tobytes()).hexdigest()
    ppe = _get(("ppe", ekey), lambda: _preprocess_edges(edge_index))

    xpad = np.zeros((NPAD, H), np.float32)
    xpad[:N] = x
    wrep = np.tile(np.asarray(w_lead, np.float32)[None, :], (P, 1))
    blead = np.full((P, 1), np.float32(b_lead), np.float32)

    # ---- launch A: logits ---------------------------------------------------
    nca = _get("la", _build_la)
    in_a = [{
        "xf": np.ascontiguousarray(
            xpad[k * NPC : (k + 1) * NPC].reshape(BPC, P, H)
            .transpose(1, 0, 2).reshape(P, BPC * H)),
        "wrep": wrep,
        "blead": blead,
    } for k in range(NCORES)]
    ra = run_bass_kernel_spmd(nca, in_a, core_ids=CORES)

    logits_full = np.zeros(NPAD, np.float32)
    for k in range(NCORES):
        lg = ra.results[k]["logits_o"]            # [P, BPC]
        logits_full[k * NPC : (k + 1) * NPC] = lg.T.reshape(-1)
    logits_full[N:] = NEG

    # ---- launch B: election -------------------------------------------------
    ncb = _get(("lb", ekey), lambda: _build_lb(ppe["WU"]))
    es = ppe["elog_src"]
    in_b = [{
        "epad": np.ascontiguousarray(
            np.where(es[k] >= 0, logits_full[np.maximum(es[k], 0)],
                     NEG).astype(np.float32)),
        "srcp1": ppe["srcp1"][k],
    } for k in range(NCORES)]
    rb = run_bass_kernel_spmd(ncb, in_b, core_ids=CORES)

    lead = np.zeros(NPAD, np.int64)
    for k in range(NCORES):
        lf = rb.results[k]["lead_o"]              # [P, BPC] f32 node ids
        lead[k * NPC : (k + 1) * NPC] = lf.T.reshape(-1).astype(np.int64)

    # ---- leader-dependent preprocessing ------------------------------------
    lkey = hashlib.md5(lead.tobytes()).hexdigest()
    ppl = _get(("ppl", ekey, lkey), lambda: _preprocess_leaders(ppe, lead))
    gsc, gsd, NB2 = ppl["gsc"], ppl["gsd"], ppl["NB2"]

    # ---- launch C: filtered mean + MLP (paired gather) ---------------------
    xbf = xpad.astype(bf16)
    w1f = np.ascontiguousarray(np.asarray(w1, np.float32).astype(bf16))
    b1c = np.ascontiguousarray(np.asarray(b1, np.float32).reshape(H, 1))
    w2f = np.ascontiguousarray(np.asarray(w2, np.float32).astype(bf16))
    b2c = np.ascontiguousarray(np.asarray(b2, np.float32).reshape(C, 1))

    structure_key = hashlib.md5(
        np.asarray(gsc["occs"], np.int64).tobytes()
        + np.int64(NB2).tobytes() + np.int64(gsc["NROWS"]).tobytes()
    ).hexdigest()
    ncc = _get(("lc", structure_key), lambda: _build_lc(gsc, NB2, NB2))
    in_c = []
    Rc = gsc["R"]
    for k in range(NCORES):
        grp = gsc["pairs"][k]
        xtab2 = np.zeros((gsc["NROWS"], Rc * H), bf16)
        for r in range(Rc):
            xtab2[: len(grp[0]), r * H : (r + 1) * H] = xbf[grp[r]]
        rcf = ppl["rc"][k].T.reshape(-1)          # [NB2*P], [b*128+d]
        in_c.append({
            "xtab2": xtab2,
            "idx16": gsc["idx16"][k],
            "amat": gsc["amat"][k],
            "rc2": np.ascontiguousarray(
                np.broadcast_to(rcf[None, :], (P, NB2 * P))),
            "w1": w1f,
            "b1": b1c,
            "w2": w2f,
            "b2c": b2c,
        })
    rc_ = run_bass_kernel_spmd(ncc, in_c, core_ids=CORES)

    # ---- assemble leader report table --------------------------------------
    tab_rows = NCORES * NB2 * P
    reptab = np.zeros((tab_rows, C), bf16)
    for k in range(NCORES):
        rp = rc_.results[k]["rep_o"]              # [C, NB2*P] bf16 transposed
        reptab[k * NB2 * P : (k + 1) * NB2 * P] = rp.T

    # ---- launch D: final gather (paired) -----------------------------------
    structure_key_d = hashlib.md5(
        np.asarray(gsd["occs"], np.int64).tobytes()
        + np.int64(gsd["NROWS"]).tobytes()).hexdigest()
    ncd = _get(("ld", structure_key_d), lambda: _build_ld(gsd))
    in_d = []
    Rd = gsd["R"]
    for k in range(NCORES):
        grp = gsd["pairs"][k]
        rt2 = np.zeros((gsd["NROWS"], Rd * C), bf16)
        for r in range(Rd):
            rt2[: len(grp[0]), r * C : (r + 1) * C] = reptab[grp[r]]
        in_d.append({
            "reptab2": rt2,
            "ldidx16": gsd["idx16"][k],
            "emat": gsd["amat"][k],
        })
    rd = run_bass_kernel_spmd(ncd, in_d, core_ids=CORES)

    out = np.zeros((N, C), np.float32)
    bo = np.asarray(gsd["block_order"], np.int64)
    for k in range(NCORES):
        g = rd.results[k]["gath_o"].reshape(P, BPC, C)
        g_true = np.empty_like(g)
        g_true[:, bo, :] = g
        node_rows = g_true.transpose(1, 0, 2).reshape(NPC, C)
        n0 = k * NPC
        n1 = min((k + 1) * NPC, N)
        out[n0:n1] = node_rows[: n1 - n0]
    return out
